# revision 8
# baseline (speedup 1.0000x reference)
"""Trainium2 Bass kernel v3: ragged GQA flash-decode attention.

Key ideas:
  - K/Q/P in bf16, V in fp8 e3m4 (error budget 2e-2; exact-input sim: 1.7e-2).
  - Flipped matmul orientation: Q (and P^T) are the stationary operands
    (4-column weight loads ~3ns) while K^T / V stream as moving operands.
  - 2-tile groups (256 slots): one contiguous DMA per group; per-seq group
    padding costs only ~2.6% extra bytes.
  - Col-tiling: scores and PV run 4 matmuls concurrently in distinct
    32-column groups of the PE array (tile_position); outputs land on
    32-aligned partition strips (walrus requires the alignment).
  - No mask tensor: host zeroes invalid K/V slots, so they contribute
    p=exp(0)=1 to l and 0 to O; host subtracts the exact count from l.
  - ACT exp with accum_out produces l for free.
  - P^T via PE transpose (identity streaming); kv DMAs keep the SP queue to
    themselves; o/l writebacks go through the idle Pool engine (SWDGE).

Layouts (per core, TG groups of GRP=2 tiles):
  kv  [TG*128, 3072] bf16-typed: group g row p, col j*1536 + c; tile j has
      K^T bf16 at c in [0,1024) ([d=p, h*128+s]) and V fp8e3m4 (bitcast,
      bf16-col units) at c in [1024,1536) ([s=p, 64h + d/2]).
  qt  [128, 32*TG] bf16: q for group g at cols 32g..32g+32, col = h*G+g.
  o   [128, 256*TG] bf16: per group two [128,128] blocks (head batch b2=0:
      heads 0-3, b2=1: heads 4-7); head h at partition strip 32(h%4)..+4,
      rows = [G], cols = d.
  l   [128, 2*TG] f32: per group 2 cols (b2); same strip layout.
"""

import math
from contextlib import ExitStack

import numpy as np

N_CORES = 8
B, HQ, HKV, D = 16, 32, 8, 128
G = HQ // HKV
ROW = 2 * HKV * D   # 2048 floats per kv_buffer row
GRP = 2             # tiles per group
TW = 1536           # bf16 cols per tile block (K 1024 + V-as-fp8 512)
GW = GRP * TW
SCALE = 1.0 / math.sqrt(D)

_COMPILED: dict = {}


def _build_program(TG: int, niter: int = 1, *, kv_bufs=6, psum_bufs=2, p_bufs=4,
                   skew=1, wb_chunk=2, v_f8=True, qt_pool=True, lo_split=True,
                   kv_qsplit=0, kv_split=0, wb_strided=0, tail_eager=2, ablate=""):
    import concourse.mybir as mybir
    import concourse.tile as tile
    from concourse import bacc
    from concourse.masks import make_identity

    f32 = mybir.dt.float32
    bf16 = mybir.dt.bfloat16
    f8e3 = mybir.dt.float8e3
    nc = bacc.Bacc("TRN2", target_bir_lowering=False, debug=False, num_devices=N_CORES)

    kv = nc.dram_tensor("kv", [TG * 128, GW], bf16, kind="ExternalInput").ap()
    qt = nc.dram_tensor("qt", [128, 32 * TG], bf16, kind="ExternalInput").ap()
    o_rows = 16 if wb_strided else 128
    o = nc.dram_tensor("o", [o_rows, 260 * TG], bf16, kind="ExternalOutput").ap()

    with tile.TileContext(nc) as tc, ExitStack() as ctx:
        kv_pool = ctx.enter_context(tc.tile_pool(name="kv", bufs=kv_bufs))
        sc_pools = [
            ctx.enter_context(tc.tile_pool(name=f"sc{i}", bufs=psum_bufs, space="PSUM"))
            for i in range(2)
        ]
        p_pools = [
            ctx.enter_context(tc.tile_pool(name=f"p{i}", bufs=p_bufs))
            for i in range(2)
        ]
        pts_pools = [
            ctx.enter_context(tc.tile_pool(name=f"pts{i}", bufs=p_bufs))
            for i in range(2)
        ]
        tp_ps_pool = ctx.enter_context(
            tc.tile_pool(name="tp", bufs=1, space="PSUM"))
        o_pool = ctx.enter_context(tc.tile_pool(name="ops", bufs=psum_bufs, space="PSUM"))
        const_pool = ctx.enter_context(tc.tile_pool(name="const", bufs=1))
        io_pool = ctx.enter_context(tc.tile_pool(name="io", bufs=1))

        o_all = io_pool.tile([128, 260 * TG], bf16)

        ident = const_pool.tile([128, 128], bf16)
        make_identity(nc, ident[:])
        if ablate:
            nc.gpsimd.memset(o_all[:], 1.0)

        qt_s = io_pool.tile([128, 32 * TG], bf16)
        (nc.gpsimd if qt_pool else nc.sync).dma_start(qt_s[:], qt)

        def v_ap(kv2, j, h):
            if v_f8:
                return kv2[:, TW * j + 1024 + 64 * h:
                           TW * j + 1024 + 64 * (h + 1)].bitcast(f8e3)
            return kv2[:, TW * j + 1024 + 128 * h:TW * j + 1024 + 128 * (h + 1)]

        def emit_pv(st):
            g, kv2, p = st
            if ablate == "sc":
                for i in range(2):
                    nc.vector.tensor_copy(
                        o_all[:, 260 * g + 128 * i:260 * g + 128 * i + 128],
                        p[i][:, 0:128])
                return
            # transpose P blocks on the PE (data loads as weights, identity
            # streams through), then one DVE copy per batch to SBUF
            pts = [pts_pools[i].tile([128, 128 * GRP], bf16, name=f"pts_{i}")
                   for i in range(2)]
            for i in range(2):
                tp = tp_ps_pool.tile([128, 128 * GRP], bf16, name=f"tp_{i}")
                for j in range(GRP):
                    nc.tensor.transpose(tp[:, 128 * j:128 * (j + 1)],
                                        p[i][:, 128 * j:128 * (j + 1)], ident[:])
                nc.vector.tensor_copy(pts[i][:], tp[:])
            O = o_pool.tile([128, 256], f32)
            for b2 in range(2):
                for j in range(GRP):
                    for c in range(4):
                        h = 4 * b2 + c
                        nc.tensor.matmul(
                            O[32 * c:32 * c + 4, 128 * b2:128 * b2 + 128],
                            pts[b2][:, 128 * j + 32 * c:128 * j + 32 * c + 4],
                            v_ap(kv2, j, h),
                            start=(j == 0), stop=(j == GRP - 1),
                            tile_position=(0, 32 * c),
                        )
            nc.vector.tensor_copy(o_all[:, 260 * g:260 * g + 256], O[:])
            if (g + 1) % wb_chunk == 0 or g == TG - 1:
                c0 = 260 * (emit_pv.wb_done)
                c1 = 260 * (g + 1)
                eng = nc.sync if g == TG - 1 else nc.gpsimd
                if wb_strided:
                    for a in range(4):
                        eng.dma_start(o[4 * a:4 * (a + 1), c0:c1],
                                      o_all[32 * a:32 * a + 4, c0:c1])
                else:
                    eng.dma_start(o[:, c0:c1], o_all[:, c0:c1])
                emit_pv.wb_done = g + 1

        def body():
            emit_pv.wb_done = 0
            pend = []
            for g in range(TG):
                kv2 = kv_pool.tile([128, GW], bf16)
                kv_eng = nc.scalar if (kv_qsplit and g % 2) else nc.sync
                if kv_split:
                    src_r = kv[g * 128:(g + 1) * 128, :].rearrange(
                        "p (j c) -> p j c", j=GRP)
                    dst_r = kv2[:].rearrange("p (j c) -> p j c", j=GRP)
                    kv_eng.dma_start(dst_r[:, :, 0:1024], src_r[:, :, 0:1024])
                    nc.gpsimd.dma_start(dst_r[:, :, 1024:TW], src_r[:, :, 1024:TW])
                else:
                    kv_eng.dma_start(kv2[:], kv[g * 128:(g + 1) * 128, :])
                if ablate == "dma":
                    nc.vector.tensor_copy(o_all[0:128, 256 * g:256 * g + 1],
                                          kv2[:, 0:1])
                    continue
                kv2r = kv2[:].rearrange("p (j c) -> p j c", j=GRP)
                sc = [sc_pools[i].tile([128, 128 * GRP], f32, name=f"sc_{i}")
                      for i in range(2)]
                for h in range(HKV):
                    b2, c = divmod(h, 4)
                    nc.tensor.matmul(
                        sc[b2][32 * c:32 * c + 4, :],
                        qt_s[:, 32 * g + 4 * h:32 * g + 4 * h + 4],
                        kv2r[:, :, 128 * h:128 * (h + 1)],
                        start=True, stop=True,
                        tile_position=(0, 32 * c),
                    )
                p = [p_pools[i].tile([128, 128 * GRP], bf16, name=f"p_{i}")
                     for i in range(2)]
                lg = o_all[:, 260 * g + 256:260 * (g + 1)].bitcast(f32)
                for i in range(2):
                    nc.scalar.activation(
                        p[i][:],
                        sc[i][:],
                        mybir.ActivationFunctionType.Exp,
                        scale=SCALE,
                        accum_out=lg[:, i:i + 1],
                    )
                pend.append((g, kv2, p))
                if len(pend) > skew:
                    emit_pv(pend.pop(0))
            for st in pend:
                emit_pv(st)

        if niter > 1:
            with tc.For_i(0, niter, 1):
                body()
        else:
            body()

    nc.compile()
    return nc


def _make_runner(nc):
    """Persistent jitted SPMD runner for a compiled Bacc program (axon path)."""
    import jax
    import concourse.mybir as mybir
    from jax.experimental.shard_map import shard_map
    from jax.sharding import Mesh, PartitionSpec

    from concourse.bass2jax import (
        _bass_exec_p,
        install_neuronx_cc_hook,
        partition_id_tensor,
    )

    install_neuronx_cc_hook()

    partition_name = nc.partition_id_tensor.name if nc.partition_id_tensor else None
    in_names, out_names, out_avals, zero_shapes = [], [], [], []
    for alloc in nc.m.functions[0].allocations:
        if not isinstance(alloc, mybir.MemoryLocationSet):
            continue
        name = alloc.memorylocations[0].name
        if alloc.kind == "ExternalInput":
            if name != partition_name:
                in_names.append(name)
        elif alloc.kind == "ExternalOutput":
            out_names.append(name)
            shape = tuple(alloc.tensor_shape)
            dtype = mybir.dt.np(alloc.dtype)
            out_avals.append(jax.core.ShapedArray(shape, dtype))
            zero_shapes.append((shape, dtype))
    n_params = len(in_names)
    n_outs = len(out_avals)
    all_in_names = list(in_names) + list(out_names)
    if partition_name is not None:
        all_in_names.append(partition_name)

    def _body(*args):
        operands = list(args)
        if partition_name is not None:
            operands.append(partition_id_tensor())
        outs = _bass_exec_p.bind(
            *operands,
            out_avals=tuple(out_avals),
            in_names=tuple(all_in_names),
            out_names=tuple(out_names),
            lowering_input_output_aliases=(),
            sim_require_finite=True,
            sim_require_nnan=True,
            nc=nc,
        )
        return tuple(outs)

    devices = jax.devices()[:N_CORES]
    assert len(devices) >= N_CORES, f"need {N_CORES} devices, have {len(devices)}"
    mesh = Mesh(np.asarray(devices[:N_CORES]), ("core",))
    in_specs = (PartitionSpec("core"),) * (n_params + n_outs)
    out_specs = (PartitionSpec("core"),) * n_outs
    donate = tuple(range(n_params, n_params + n_outs))
    sharded = jax.jit(
        shard_map(
            _body, mesh=mesh, in_specs=in_specs, out_specs=out_specs, check_rep=False
        ),
        donate_argnums=donate,
        keep_unused=True,
    )

    def run(concat_inputs):
        args = [concat_inputs[name] for name in in_names]
        zeros = [
            np.zeros((N_CORES * s[0], *s[1:]), d) for (s, d) in zero_shapes
        ]
        out_arrs = sharded(*args, *zeros)
        out_arrs = [np.asarray(a) for a in out_arrs]
        return {name: out_arrs[i] for i, name in enumerate(out_names)}

    run.in_names = in_names
    run.out_names = out_names
    run.out_avals = out_avals
    run.zero_shapes = zero_shapes
    run.sharded = sharded
    run.mesh = mesh
    return run


def _plan(b_seq_len):
    """Group list [(b, jg)] padded to 8*TG; each group covers GRP tile slots.

    Returns (groups, TG, n_invalid) where n_invalid[b] = ngroups_b*GRP*128 - len_b.
    """
    lens = [int(x) for x in b_seq_len]
    groups = []
    n_invalid = np.zeros(B, dtype=np.int64)
    for b, ln in enumerate(lens):
        ntiles = (ln + 127) // 128
        ng = (ntiles + GRP - 1) // GRP
        n_invalid[b] = ng * (GRP * 128) - ln
        for jg in range(ng):
            groups.append((b, jg))
    TG = (len(groups) + N_CORES - 1) // N_CORES
    groups += [(-1, -1)] * (N_CORES * TG - len(groups))
    return groups, TG, n_invalid


def _pack(xq, xk, xv, kv_buffer, cur_select_index, start_index, b_seq_len,
          groups, TG, v_f8=True):
    import ml_dtypes

    bf = ml_dtypes.bfloat16
    e3 = ml_dtypes.float8_e3m4
    lens = np.asarray(b_seq_len, dtype=np.int64)
    starts = np.asarray(start_index, dtype=np.int64)
    csi = np.asarray(cur_select_index, dtype=np.int64)
    kvb = np.asarray(kv_buffer).reshape(-1, ROW)
    new_kv = np.concatenate(
        [np.asarray(xk)[:, 0], np.asarray(xv)[:, 0]], axis=1
    ).reshape(B, ROW)
    q_bf = np.asarray(xq)[:, 0].transpose(0, 2, 1).astype(bf)  # [B, D, HQ]

    ng_all = N_CORES * TG
    ent = []
    for b, jg in groups:
        for u in range(GRP):
            j = jg * GRP + u
            if b >= 0 and j * 128 < int(lens[b]):
                ent.append((b, j))
            else:
                ent.append((-1, -1))
    ent = np.array(ent, dtype=np.int64)  # [ng_all*GRP, 2]
    real = np.nonzero(ent[:, 0] >= 0)[0]
    eb, ej = ent[real, 0], ent[real, 1]

    rows = (starts[eb] + ej * 128)[:, None] + np.arange(128)[None, :]
    arr = kvb[rows]  # [nreal, 128, 2048] f32 gather
    nvalid = np.minimum(128, lens[eb] - ej * 128)
    mask = np.arange(128)[None, :] >= nvalid[:, None]
    arr[mask] = 0.0
    sel = np.nonzero((csi[eb] >= rows[:, 0]) & (csi[eb] < rows[:, 0] + 128))[0]
    arr[sel, (csi[eb[sel]] - rows[sel, 0])] = new_kv[eb[sel]]

    kt = (
        arr[:, :, :HKV * D].reshape(-1, 128, HKV, D)
        .transpose(0, 3, 2, 1).reshape(-1, 128, HKV * 128)
    )
    v = arr[:, :, HKV * D:]
    tiles = np.zeros((ng_all * GRP, 128, 2 * TW), dtype=np.uint8)
    tiles[real, :, :2048] = kt.astype(bf).view(np.uint8)
    if v_f8:
        tiles[real, :, 2048:3072] = v.astype(e3).view(np.uint8)
    else:
        tiles[real, :, 2048:] = v.astype(bf).view(np.uint8)
    kv_all = (
        tiles.reshape(ng_all, GRP, 128, 2 * TW)
        .transpose(0, 2, 1, 3).reshape(ng_all * 128, GRP * 2 * TW)
        .view(bf)
    )

    qt_all = np.zeros((N_CORES, 128, 32 * TG), dtype=bf)
    for i, (b, jg) in enumerate(groups):
        if b < 0:
            continue
        c, s = divmod(i, TG)
        qt_all[c, :, 32 * s:32 * s + 32] = q_bf[b]
    return {"kv": kv_all, "qt": qt_all.reshape(N_CORES * 128, 32 * TG)}


def _combine(o_cat, groups, TG, n_invalid):
    acc = np.zeros((B, HQ, D), dtype=np.float64)
    lacc = np.zeros((B, HQ), dtype=np.float64)
    nrows = o_cat.size // (N_CORES * 260 * TG)
    o_cat = o_cat.reshape(N_CORES, nrows, 260 * TG)
    if nrows == 128:  # legacy layout: strips at partitions 32c+g
        o_cat = o_cat.reshape(N_CORES, 4, 32, 260 * TG)[:, :, 0:4]
        o_cat = o_cat.reshape(N_CORES, 16, 260 * TG)
    l_cat = np.ascontiguousarray(
        o_cat.reshape(N_CORES, 16, TG, 260)[:, :, :, 256:260]
    ).view(np.float32)  # [NC, 16, TG, 2]
    o_f = o_cat.astype(np.float64)
    for i, (b, jg) in enumerate(groups):
        if b < 0:
            continue
        c, s = divmod(i, TG)
        for b2 in range(2):
            Ob = o_f[c][:, 260 * s + 128 * b2:260 * s + 128 * (b2 + 1)]
            lb = l_cat[c, :, s, b2]
            acc[b, 16 * b2:16 * (b2 + 1)] += Ob
            lacc[b, 16 * b2:16 * (b2 + 1)] += lb
    lacc -= n_invalid[:, None]
    out = acc / lacc[:, :, None]
    return out.reshape(B, 1, HQ * D).astype(np.float32)


def get_compiled(TG, niter=1):
    key = (TG, niter)
    if key not in _COMPILED:
        nc = _build_program(TG, niter)
        _COMPILED[key] = _make_runner(nc)
    return _COMPILED[key]


def kernel(xq, xk, xv, kv_buffer, cur_select_index, start_index, b_seq_len,
           max_actual_seq_len=None):
    groups, TG, n_invalid = _plan(b_seq_len)
    inputs = _pack(xq, xk, xv, kv_buffer, cur_select_index, start_index,
                   b_seq_len, groups, TG)
    run = get_compiled(TG)
    outs = run(inputs)
    return _combine(outs["o"], groups, TG, n_invalid)
